# revision 6
# baseline (speedup 1.0000x reference)
"""CrossAttentionMemory kernel for 8 Trainium2 NeuronCores.

Reference computation (B=2, N=512, M=2048, D=H=4096):
    xq = inputs @ wq^T            [B, N, H]
    mk = memory @ wk^T            [B, M, H]
    s  = (xq @ mk^T) / sqrt(H)    [B, N, M]
    p  = softmax(s, f32) -> bf16
    out = p @ memory              [B, N, D]
    hist = seg_num - argmax(p, axis=2)  (flattened), browse = hist[0] < 4

Sharding: core c handles batch b=c//4 and memory rows j*512:(j+1)*512 (j=c%4).
Each core computes its local raw scores [512, 512] (f32) and the local
exp-weighted partial output sum [512, 4096] (f32, stabilized with the local
row max). The host combines shards flash-style and computes hist/browse from
the gathered f32 scores.

All device matmuls use contraction-major operands produced by host-side
numpy transposes (inputs.T, memory_slice.T, wq.T, wk.T), so no DMA
transposes are needed; the only on-device transposes are cheap 128x128 PE
transposes of intermediates (xq, mk, p).
"""

import sys

if "/opt/trn_rl_repo" not in sys.path:
    sys.path.insert(0, "/opt/trn_rl_repo")

import numpy as np
import ml_dtypes

B = 2
N = 512
M = 2048
D = 4096
H = 4096
P = 128
MLOC = M // 4  # 512 memory rows per core
NCORES = 8

BF16 = ml_dtypes.bfloat16

_CACHE = {}


def _build_program():
    import concourse.bacc as bacc
    import concourse.mybir as mybir
    import concourse.tile as tile
    from concourse.masks import make_identity

    fp32 = mybir.dt.float32
    bf16 = mybir.dt.bfloat16

    nc = bacc.Bacc("TRN2", target_bir_lowering=False)

    xin_t = nc.dram_tensor("xin_t", [D, N], bf16, kind="ExternalInput")
    mem_t = nc.dram_tensor("mem_t", [D, MLOC], bf16, kind="ExternalInput")
    mem_n = nc.dram_tensor("mem_n", [MLOC, D], bf16, kind="ExternalInput")
    wq_t = nc.dram_tensor("wq_t", [D, H], bf16, kind="ExternalInput")
    wk_t = nc.dram_tensor("wk_t", [D, H], bf16, kind="ExternalInput")

    out_part = nc.dram_tensor("out_part", [N, D], fp32, kind="ExternalOutput")
    scores_raw = nc.dram_tensor("scores_raw", [N, MLOC], fp32, kind="ExternalOutput")

    KC = D // P  # 32 contraction chunks for the projections
    HS = H // 512  # 8 output column slices for the projections
    NCH = N // P  # 4 query-row chunks
    MCH = MLOC // P  # 4 memory-row chunks
    DS = D // 512  # 8 output column slices for the final matmul

    with tile.TileContext(nc) as tc:
        with (
            tc.tile_pool(name="const", bufs=1) as const,
            tc.tile_pool(name="act_in", bufs=1) as act_in,
            tc.tile_pool(name="wstream", bufs=6) as wstream,
            tc.tile_pool(name="psum_acc", bufs=6, space="PSUM") as psum_acc,
            tc.tile_pool(name="psum_tr", bufs=2, space="PSUM") as psum_tr,
            tc.tile_pool(name="evict", bufs=4) as evict,
            tc.tile_pool(name="hmajor", bufs=1) as hmajor,
            tc.tile_pool(name="soft", bufs=3) as soft,
            tc.tile_pool(name="stats", bufs=8) as stats,
            tc.tile_pool(name="outev", bufs=4) as outev,
        ):
            ident = const.tile([P, P], bf16)
            make_identity(nc, ident)

            # resident activation inputs (contraction-major)
            xin_sb = act_in.tile([P, KC, N], bf16, tag="xin")
            memt_sb = act_in.tile([P, KC, MLOC], bf16, tag="memt")
            memn_sb = act_in.tile([P, MCH, D], bf16, tag="memn")
            for k in range(KC):
                nc.sync.dma_start(out=xin_sb[:, k, :], in_=xin_t[k * P:(k + 1) * P, :])
                nc.sync.dma_start(out=memt_sb[:, k, :], in_=mem_t[k * P:(k + 1) * P, :])
            for mc in range(MCH):
                nc.sync.dma_start(
                    out=memn_sb[:, mc, :], in_=mem_n[mc * P:(mc + 1) * P, :]
                )

            # H-major projected activations
            xqT = hmajor.tile([P, KC, N], bf16, tag="xqT")
            mkT = hmajor.tile([P, KC, MLOC], bf16, tag="mkT")

            def project(w_dram, act_sb, act_cols, dst):
                # dst[h, n] = sum_d w[h, d] * act[n, d], written H-major as
                # dst[:, h_chunk, n]; act_sb is [P, KC, act_cols] d-major.
                nch = act_cols // P
                for hs in range(HS):
                    psums = [
                        psum_acc.tile([P, 512], fp32, tag="pacc", name="pacc")
                        for _ in range(nch)
                    ]
                    for k in range(KC):
                        wt = wstream.tile([P, 512], bf16)
                        nc.sync.dma_start(
                            out=wt[:],
                            in_=w_dram[k * P:(k + 1) * P, hs * 512:(hs + 1) * 512],
                        )
                        for ni in range(nch):
                            nc.tensor.matmul(
                                psums[ni][:],
                                lhsT=act_sb[:, k, ni * P:(ni + 1) * P],
                                rhs=wt[:],
                                start=(k == 0),
                                stop=(k == KC - 1),
                            )
                    for ni in range(nch):
                        ev = evict.tile([P, 512], bf16)
                        nc.scalar.copy(ev[:], psums[ni][:])
                        for t in range(4):
                            pt = psum_tr.tile([P, P], bf16)
                            nc.tensor.transpose(
                                pt[:], ev[:, t * P:(t + 1) * P], ident[:]
                            )
                            nc.vector.tensor_copy(
                                dst[:, hs * 4 + t, ni * P:(ni + 1) * P], pt[:]
                            )

            project(wq_t, xin_sb, N, xqT)
            project(wk_t, memt_sb, MLOC, mkT)

            # scores + softmax numerator + p^T
            pT = hmajor.tile([P, MCH, N], bf16, tag="pT")
            for ni in range(NCH):
                ps = psum_acc.tile([P, MLOC], fp32, tag="pacc")
                for k in range(KC):
                    nc.tensor.matmul(
                        ps[:],
                        lhsT=xqT[:, k, ni * P:(ni + 1) * P],
                        rhs=mkT[:, k, :],
                        start=(k == 0),
                        stop=(k == KC - 1),
                    )
                # The reference materializes the scores einsum in bf16, so
                # quantize raw scores to bf16 before the softmax chain; the
                # host performs the identical quantization on scores_raw.
                sc = soft.tile([P, MLOC], fp32, tag="sc")
                nc.vector.tensor_copy(sc[:], ps[:])
                nc.sync.dma_start(
                    out=scores_raw[ni * P:(ni + 1) * P, :], in_=sc[:]
                )
                sb = soft.tile([P, MLOC], bf16, tag="sb")
                nc.vector.tensor_copy(sb[:], ps[:])
                mx = stats.tile([P, 1], fp32, tag="mx")
                nc.vector.reduce_max(mx[:], sb[:], axis=mybir.AxisListType.X)
                nb = stats.tile([P, 1], fp32, tag="nb")
                nc.scalar.mul(nb[:], mx[:], -1.0 / 64.0)
                pb = soft.tile([P, MLOC], bf16, tag="pb")
                nc.scalar.activation(
                    pb[:],
                    sb[:],
                    mybir.ActivationFunctionType.Exp,
                    bias=nb[:],
                    scale=1.0 / 64.0,
                )
                for mc in range(MCH):
                    pt = psum_tr.tile([P, P], bf16)
                    nc.tensor.transpose(
                        pt[:], pb[:, mc * P:(mc + 1) * P], ident[:]
                    )
                    nc.vector.tensor_copy(
                        pT[:, mc, ni * P:(ni + 1) * P], pt[:]
                    )

            # out_part = p @ mem_slice (f32 partials)
            for ni in range(NCH):
                for dsi in range(DS):
                    po = psum_acc.tile([P, 512], fp32, tag="pacc")
                    for mc in range(MCH):
                        nc.tensor.matmul(
                            po[:],
                            lhsT=pT[:, mc, ni * P:(ni + 1) * P],
                            rhs=memn_sb[:, mc, dsi * 512:(dsi + 1) * 512],
                            start=(mc == 0),
                            stop=(mc == MCH - 1),
                        )
                    oe = outev.tile([P, 512], fp32)
                    nc.scalar.copy(oe[:], po[:])
                    nc.sync.dma_start(
                        out=out_part[ni * P:(ni + 1) * P, dsi * 512:(dsi + 1) * 512],
                        in_=oe[:],
                    )

    nc.compile()
    return nc


def _get_program():
    if "nc" not in _CACHE:
        _CACHE["nc"] = _build_program()
    return _CACHE["nc"]


def _as_bf16(x):
    x = np.asarray(x)
    if x.dtype != BF16:
        x = x.astype(BF16)
    return x


def kernel(memory, inputs, wq, wk, seg_num, _want_results=False):
    from concourse.bass_utils import run_bass_kernel_spmd

    memory = _as_bf16(memory)
    inputs = _as_bf16(inputs)
    wq = _as_bf16(wq)
    wk = _as_bf16(wk)
    seg = int(np.asarray(seg_num))

    nc = _get_program()

    wq_t = np.ascontiguousarray(wq.T)
    wk_t = np.ascontiguousarray(wk.T)
    in_maps = []
    for c in range(NCORES):
        b, j = divmod(c, 4)
        mem_slice = memory[b, j * MLOC:(j + 1) * MLOC]
        in_maps.append(
            {
                "xin_t": np.ascontiguousarray(inputs[b].T),
                "mem_t": np.ascontiguousarray(mem_slice.T),
                "mem_n": np.ascontiguousarray(mem_slice),
                "wq_t": wq_t,
                "wk_t": wk_t,
            }
        )

    res = run_bass_kernel_spmd(nc, in_maps, list(range(NCORES)))

    output = np.empty((B, N, D), dtype=np.float32)
    hist_parts = []
    for b in range(B):
        raw = np.concatenate(
            [res.results[4 * b + j]["scores_raw"] for j in range(4)], axis=1
        ).astype(np.float32)  # [N, M]
        # mirror the reference: the scores einsum materializes in bf16
        logits = (raw * np.float32(1.0 / 64.0)).astype(BF16).astype(np.float32)
        m_glob = logits.max(axis=1)
        l_glob = np.exp(logits - m_glob[:, None]).sum(axis=1)
        acc = np.zeros((N, D), dtype=np.float32)
        for j in range(4):
            m_loc = logits[:, j * MLOC:(j + 1) * MLOC].max(axis=1)
            scale = np.exp(m_loc - m_glob)
            acc += res.results[4 * b + j]["out_part"] * scale[:, None]
        output[b] = acc / l_glob[:, None]
        am = np.argmax(logits, axis=1)
        hist_parts.append((seg - am).astype(np.int32))

    hist = np.concatenate(hist_parts)
    browse = np.bool_(hist[0] < 4)
    out = (output.astype(BF16), hist, browse)
    if _want_results:
        return out, res
    return out


# revision 12
# speedup vs baseline: 1.0319x; 1.0319x over previous
"""CrossAttentionMemory kernel for 8 Trainium2 NeuronCores.

Reference computation (B=2, N=512, M=2048, D=H=4096):
    xq = inputs @ wq^T            [B, N, H]
    mk = memory @ wk^T            [B, M, H]
    s  = (xq @ mk^T) / sqrt(H)    [B, N, M]
    p  = softmax(s, f32) -> bf16
    out = p @ memory              [B, N, D]
    hist = seg_num - argmax(p, axis=2)  (flattened), browse = hist[0] < 4

Sharding: core c handles batch b=c//4 and memory rows j*512:(j+1)*512 (j=c%4).
Each core computes its local raw scores [512, 512] (f32) and the local
exp-weighted partial output sum [512, 4096] (f32, stabilized with the local
row max). The host combines shards flash-style and computes hist/browse from
the gathered f32 scores.

All device matmuls use contraction-major operands produced by host-side
numpy transposes (inputs.T, memory_slice.T, wq.T, wk.T), so no DMA
transposes are needed; the only on-device transposes are cheap 128x128 PE
transposes of intermediates (xq, mk, p).
"""

import sys

if "/opt/trn_rl_repo" not in sys.path:
    sys.path.insert(0, "/opt/trn_rl_repo")

import numpy as np
import ml_dtypes

B = 2
N = 512
M = 2048
D = 4096
H = 4096
P = 128
MLOC = M // 4  # 512 memory rows per core
NCORES = 8

BF16 = ml_dtypes.bfloat16

_CACHE = {}


def _build_program():
    import concourse.bacc as bacc
    import concourse.mybir as mybir
    import concourse.tile as tile
    from concourse.masks import make_identity

    fp32 = mybir.dt.float32
    bf16 = mybir.dt.bfloat16

    nc = bacc.Bacc("TRN2", target_bir_lowering=False)

    KC_ = D // P
    HS_ = H // 512
    xin_t = nc.dram_tensor("xin_t", [D, N], bf16, kind="ExternalInput")
    mem_t = nc.dram_tensor("mem_t", [D, MLOC], bf16, kind="ExternalInput")
    mem_n = nc.dram_tensor("mem_n", [MLOC, D], bf16, kind="ExternalInput")
    # weights pre-tiled on host: [hs, k, 128, 512] so each (hs, k) weight
    # tile is one contiguous 128KB DMA
    wq_t = nc.dram_tensor("wq_t", [HS_, KC_, P, 512], bf16, kind="ExternalInput")
    wk_t = nc.dram_tensor("wk_t", [HS_, KC_, P, 512], bf16, kind="ExternalInput")

    out_part = nc.dram_tensor("out_part", [N, D], fp32, kind="ExternalOutput")
    scores_raw = nc.dram_tensor("scores_raw", [N, MLOC], fp32, kind="ExternalOutput")

    KC = D // P  # 32 contraction chunks for the projections
    HS = H // 512  # 8 output column slices for the projections
    NCH = N // P  # 4 query-row chunks
    MCH = MLOC // P  # 4 memory-row chunks
    DS = D // 512  # 8 output column slices for the final matmul

    with tile.TileContext(nc) as tc:
        with (
            tc.tile_pool(name="const", bufs=1) as const,
            tc.tile_pool(name="act_in", bufs=1) as act_in,
            tc.tile_pool(name="wstream", bufs=6) as wstream,
            tc.tile_pool(name="psum_acc", bufs=6, space="PSUM") as psum_acc,
            tc.tile_pool(name="psum_tr", bufs=2, space="PSUM") as psum_tr,
            tc.tile_pool(name="evict", bufs=4) as evict,
            tc.tile_pool(name="hmajor", bufs=1) as hmajor,
            tc.tile_pool(name="soft", bufs=3) as soft,
            tc.tile_pool(name="stats", bufs=8) as stats,
            tc.tile_pool(name="outev", bufs=4) as outev,
        ):
            ident = const.tile([P, P], bf16)
            make_identity(nc, ident)

            # resident activation inputs (contraction-major); load each
            # right before the phase that consumes it so the weight
            # stream isn't queued behind preloads
            xin_sb = act_in.tile([P, KC, N], bf16, tag="xin")
            memt_sb = act_in.tile([P, KC, MLOC], bf16, tag="memt")
            memn_sb = act_in.tile([P, MCH, D], bf16, tag="memn")
            for k in range(KC):
                nc.sync.dma_start(out=xin_sb[:, k, :], in_=xin_t[k * P:(k + 1) * P, :])

            # H-major projected activations
            xqT = hmajor.tile([P, KC, N], bf16, tag="xqT")
            mkT = hmajor.tile([P, KC, MLOC], bf16, tag="mkT")

            def project(w_dram, act_sb, act_cols, dst):
                # dst[h, n] = sum_d w[h, d] * act[n, d], written H-major as
                # dst[:, h_chunk, n]; act_sb is [P, KC, act_cols] d-major.
                nch = act_cols // P
                for hs in range(HS):
                    psums = [
                        psum_acc.tile([P, 512], fp32, tag="pacc", name="pacc")
                        for _ in range(nch)
                    ]
                    for k in range(KC):
                        wt = wstream.tile([P, 512], bf16)
                        nc.sync.dma_start(out=wt[:], in_=w_dram[hs, k])
                        for ni in range(nch):
                            nc.tensor.matmul(
                                psums[ni][:],
                                lhsT=act_sb[:, k, ni * P:(ni + 1) * P],
                                rhs=wt[:],
                                start=(k == 0),
                                stop=(k == KC - 1),
                            )
                    for ni in range(nch):
                        ev = evict.tile([P, 512], bf16)
                        nc.scalar.copy(ev[:], psums[ni][:])
                        for t in range(4):
                            pt = psum_tr.tile([P, P], bf16)
                            nc.tensor.transpose(
                                pt[:], ev[:, t * P:(t + 1) * P], ident[:]
                            )
                            nc.vector.tensor_copy(
                                dst[:, hs * 4 + t, ni * P:(ni + 1) * P], pt[:]
                            )

            project(wq_t, xin_sb, N, xqT)
            for k in range(KC):
                nc.sync.dma_start(
                    out=memt_sb[:, k, :], in_=mem_t[k * P:(k + 1) * P, :]
                )
            project(wk_t, memt_sb, MLOC, mkT)
            for mc in range(MCH):
                nc.sync.dma_start(
                    out=memn_sb[:, mc, :], in_=mem_n[mc * P:(mc + 1) * P, :]
                )

            # scores + softmax numerator + p^T
            pT = hmajor.tile([P, MCH, N], bf16, tag="pT")
            for ni in range(NCH):
                ps = psum_acc.tile([P, MLOC], fp32, tag="pacc")
                for k in range(KC):
                    nc.tensor.matmul(
                        ps[:],
                        lhsT=xqT[:, k, ni * P:(ni + 1) * P],
                        rhs=mkT[:, k, :],
                        start=(k == 0),
                        stop=(k == KC - 1),
                    )
                # The reference materializes the scores einsum in bf16, so
                # quantize raw scores to bf16 before the softmax chain; the
                # host performs the identical quantization on scores_raw.
                sc = soft.tile([P, MLOC], fp32, tag="sc")
                nc.vector.tensor_copy(sc[:], ps[:])
                nc.sync.dma_start(
                    out=scores_raw[ni * P:(ni + 1) * P, :], in_=sc[:]
                )
                sb = soft.tile([P, MLOC], bf16, tag="sb")
                nc.vector.tensor_copy(sb[:], ps[:])
                mx = stats.tile([P, 1], fp32, tag="mx")
                nc.vector.reduce_max(mx[:], sb[:], axis=mybir.AxisListType.X)
                nb = stats.tile([P, 1], fp32, tag="nb")
                nc.scalar.mul(nb[:], mx[:], -1.0 / 64.0)
                pb = soft.tile([P, MLOC], bf16, tag="pb")
                nc.scalar.activation(
                    pb[:],
                    sb[:],
                    mybir.ActivationFunctionType.Exp,
                    bias=nb[:],
                    scale=1.0 / 64.0,
                )
                for mc in range(MCH):
                    pt = psum_tr.tile([P, P], bf16)
                    nc.tensor.transpose(
                        pt[:], pb[:, mc * P:(mc + 1) * P], ident[:]
                    )
                    nc.vector.tensor_copy(
                        pT[:, mc, ni * P:(ni + 1) * P], pt[:]
                    )

            # out_part = p @ mem_slice (f32 partials)
            for ni in range(NCH):
                for dsi in range(DS):
                    po = psum_acc.tile([P, 512], fp32, tag="pacc")
                    for mc in range(MCH):
                        nc.tensor.matmul(
                            po[:],
                            lhsT=pT[:, mc, ni * P:(ni + 1) * P],
                            rhs=memn_sb[:, mc, dsi * 512:(dsi + 1) * 512],
                            start=(mc == 0),
                            stop=(mc == MCH - 1),
                        )
                    oe = outev.tile([P, 512], fp32)
                    nc.scalar.copy(oe[:], po[:])
                    nc.sync.dma_start(
                        out=out_part[ni * P:(ni + 1) * P, dsi * 512:(dsi + 1) * 512],
                        in_=oe[:],
                    )

    nc.compile()
    return nc


def _get_program():
    if "nc" not in _CACHE:
        _CACHE["nc"] = _build_program()
    return _CACHE["nc"]


def _as_bf16(x):
    x = np.asarray(x)
    if x.dtype != BF16:
        x = x.astype(BF16)
    return x


def make_in_maps(memory, inputs, wq, wk):
    memory = _as_bf16(memory)
    inputs = _as_bf16(inputs)
    wq = _as_bf16(wq)
    wk = _as_bf16(wk)

    def retile_w(w):
        # w is [H, D]; device wants w.T = [D, H] pre-tiled as
        # [hs, k, 128, 512] with contiguous (hs, k) tiles
        wt = np.ascontiguousarray(w.T).reshape(D // P, P, H // 512, 512)
        return np.ascontiguousarray(wt.transpose(2, 0, 1, 3))

    wq_t = retile_w(wq)
    wk_t = retile_w(wk)
    in_maps = []
    for c in range(NCORES):
        b, j = divmod(c, 4)
        mem_slice = memory[b, j * MLOC:(j + 1) * MLOC]
        in_maps.append(
            {
                "xin_t": np.ascontiguousarray(inputs[b].T),
                "mem_t": np.ascontiguousarray(mem_slice.T),
                "mem_n": np.ascontiguousarray(mem_slice),
                "wq_t": wq_t,
                "wk_t": wk_t,
            }
        )
    return in_maps


def kernel(memory, inputs, wq, wk, seg_num, _want_results=False):
    from concourse.bass_utils import run_bass_kernel_spmd

    seg = int(np.asarray(seg_num))
    nc = _get_program()
    in_maps = make_in_maps(memory, inputs, wq, wk)
    res = run_bass_kernel_spmd(nc, in_maps, list(range(NCORES)))

    output = np.empty((B, N, D), dtype=np.float32)
    hist_parts = []
    for b in range(B):
        raw = np.concatenate(
            [res.results[4 * b + j]["scores_raw"] for j in range(4)], axis=1
        ).astype(np.float32)  # [N, M]
        # mirror the reference: the scores einsum materializes in bf16
        logits = (raw * np.float32(1.0 / 64.0)).astype(BF16).astype(np.float32)
        m_glob = logits.max(axis=1)
        l_glob = np.exp(logits - m_glob[:, None]).sum(axis=1)
        acc = np.zeros((N, D), dtype=np.float32)
        for j in range(4):
            m_loc = logits[:, j * MLOC:(j + 1) * MLOC].max(axis=1)
            scale = np.exp(m_loc - m_glob)
            acc += res.results[4 * b + j]["out_part"] * scale[:, None]
        output[b] = acc / l_glob[:, None]
        am = np.argmax(logits, axis=1)
        hist_parts.append((seg - am).astype(np.int32))

    hist = np.concatenate(hist_parts)
    browse = np.bool_(hist[0] < 4)
    out = (output.astype(BF16), hist, browse)
    if _want_results:
        return out, res
    return out
